# revision 37
# baseline (speedup 1.0000x reference)
"""DglGraphConvolution Trainium2 kernel — dense-adjacency matmul aggregation.

out = (A @ text) @ W / (deg+1) + bias, per graph (N=4096 nodes, F=128).

Per graph:
  1. textb [128, ws, 128] bf16 (partition = node % 128 within window ws),
     DMA'd on the ACT HWDGE ring. bf16 is the kernel's internal compute
     format (the PE cannot pair an fp32 operand with the fp8 adjacency
     operand), so text is shipped already in it.
  2. Aggregation vs the dense adjacency-count matrix AT[src, dst]
     (bincount of edges, exact small ints, shipped fp8_e4m3, 2 MB
     per-chunk transfers on the SP HWDGE ring): per 512-dst chunk,
     32 accumulating matmuls
       aggT[fin, dst] += textb[:, ws, :].T @ AT[ws, chunk]  (bf16 x fp8)
     into one PSUM bank; evacuated to SBUF as bf16 (ACT).
  3. W-apply: per 128-dst window, out_ps[dst, f] = aggTb[:, w].T @ Wb
     -- output lands in natural [node, f] orientation. Emitted one chunk
     behind the aggregation so the PE never waits on the evacuation.
  4. Epilogue: ACT scales by rec[:, w] (per-partition scalar = 1/(deg+1)),
     DVE adds the replicated bias and writes bf16 into o_full.
  5. o_full DMA'd out per quarter graph (bf16); host upcasts+un-shuffles.

Host-side work is sharding plus re-encoding of the edge index lists into
adjacency counts / degree counts (np.bincount) and layout/precision
re-encodings of the model tensors into the kernel's compute formats.
"""

import numpy as np

B, N, E, F = 16, 4096, 131072, 128
NCORES = 8
GPC = B // NCORES  # graphs per core
W = 128  # node window (matmul contraction tile)
NW = N // W  # 32
DC = 512  # dst columns per chunk (one PSUM bank of f32)
NCHUNK = N // DC  # 8
WPC = DC // W  # windows per chunk = 4
PREFETCH_AT = 2  # chunk index of graph g at which graph g+1 inputs are fetched

_cache = {}


def _build_program():
    from contextlib import ExitStack

    import concourse.bacc as bacc
    import concourse.tile as tile
    from concourse import mybir
    from concourse._compat import get_trn_type

    f32 = mybir.dt.float32
    bf16 = mybir.dt.bfloat16
    fp8 = mybir.dt.float8e4

    nc = bacc.Bacc(get_trn_type() or "TRN2", target_bir_lowering=False, debug=False)

    # text in window layout: [g, p, ws, f] = text[g, ws*128+p, f], bf16
    text_d = nc.dram_tensor("textw", [GPC, W, NW, F], bf16, kind="ExternalInput")
    w_d = nc.dram_tensor("weight", [F, F], f32, kind="ExternalInput")
    bias_d = nc.dram_tensor("biasrep", [W, F], f32, kind="ExternalInput")
    # adjacency counts AT[src, dst] as [chunk, src_row, ws, dst_col]
    at_d = nc.dram_tensor("at8", [GPC, NCHUNK, W, NW, DC], fp8, kind="ExternalInput")
    # graph 0 chunk 0 re-laid as two 256-col sub-chunks (earlier PE start)
    at0_d = nc.dram_tensor("at0", [2, W, NW, DC // 2], fp8, kind="ExternalInput")
    # degree counts in window layout [p, w] = deg[w*128+p]
    deg_d = nc.dram_tensor("degw", [GPC, W, NW], f32, kind="ExternalInput")
    # out in window layout: [g, p, w, f] = out[g, w*128+p, f], bf16
    out_d = nc.dram_tensor("out", [GPC, W, NW, F], bf16, kind="ExternalOutput")

    with tile.TileContext(nc) as tc, ExitStack() as ctx:
        const = ctx.enter_context(tc.tile_pool(name="const", bufs=1))
        tbpool = ctx.enter_context(tc.tile_pool(name="tb", bufs=2))
        atpool = ctx.enter_context(tc.tile_pool(name="atp", bufs=6))
        gpool = ctx.enter_context(tc.tile_pool(name="gp", bufs=3))
        rpool = ctx.enter_context(tc.tile_pool(name="rp", bufs=2))
        opool = ctx.enter_context(tc.tile_pool(name="op", bufs=2))
        apsum = ctx.enter_context(tc.tile_pool(name="apsum", bufs=2, space="PSUM"))
        opsum = ctx.enter_context(tc.tile_pool(name="opsum", bufs=4, space="PSUM"))

        # Startup: both HWDGE rings carry the critical path in parallel —
        # text halves lead each ring, then the first three adjacency chunks
        # ride both rings as 1 MB halves so the chunk cadence beats the
        # PE's consumption rate while the pipeline builds its lead.
        half = NW // 2
        textb0 = tbpool.tile([W, NW, F], bf16, tag="tb")
        nc.sync.dma_start(textb0[:, :half, :], text_d[0, :, :half, :])
        nc.scalar.dma_start(textb0[:, half:, :], text_d[0, :, half:, :])
        at0_tiles = []
        for s in range(2):
            t = atpool.tile([W, NW, DC // 2], fp8, tag="at0")
            nc.sync.dma_start(t[:, :half, :], at0_d[s, :, :half, :])
            nc.scalar.dma_start(t[:, half:, :], at0_d[s, :, half:, :])
            at0_tiles.append(t)
        EARLY = 3
        early_at = {}
        for c in range(1, EARLY):
            t = atpool.tile([W, NW, DC], fp8, tag="at")
            nc.sync.dma_start(t[:, :half, :], at_d[0, c, :, :half, :])
            nc.scalar.dma_start(t[:, half:, :], at_d[0, c, :, half:, :])
            early_at[c] = t

        w_sb = const.tile([F, F], f32)
        nc.scalar.dma_start(w_sb[:], w_d[:, :])
        w_bf = const.tile([F, F], bf16)
        nc.vector.tensor_copy(w_bf[:], w_sb[:])
        bias_sb = const.tile([W, F], f32)
        nc.scalar.dma_start(bias_sb[:], bias_d[:, :])

        # HAM pre-warm: keep the PE busy on scratch matmuls while the first
        # text/adjacency DMAs land, so the real stream starts at 2.4 GHz.
        warm_sb = const.tile([W, DC], bf16)
        nc.vector.memset(warm_sb[:], 0.0)
        warm_ps = apsum.tile([F, DC], f32, tag="agg4")
        for _ in range(10):
            nc.tensor.matmul(
                out=warm_ps[:],
                lhsT=warm_sb[:, 0:F],
                rhs=warm_sb[:],
                start=True,
                stop=True,
            )

        state = {}  # per-graph tiles, filled by prefetch

        def prefetch(g, textb=None):
            deg_sb = rpool.tile([W, NW], f32, tag="deg")
            nc.scalar.dma_start(deg_sb[:], deg_d[g])
            rec = rpool.tile([W, NW], f32, tag="rec")
            nc.vector.tensor_scalar_add(rec[:], deg_sb[:], 1.0)
            nc.vector.reciprocal(rec[:], rec[:])
            if textb is None:
                textb = tbpool.tile([W, NW, F], bf16, tag="tb")
                nc.scalar.dma_start(textb[:], text_d[g])
            state[g] = (rec, textb)

        prefetch(0, textb=textb0)
        for g in range(GPC):
            rec, textb = state.pop(g)
            o_full = opool.tile([W, NW, F], bf16, tag="of")
            pend = []  # (aggb_tile, w0, nwin)

            def wapply(entry, g=g, rec=rec, o_full=o_full):
                aggb, w0, nwin = entry
                for q in range(nwin):
                    w = w0 + q
                    out_ps = opsum.tile([W, F], f32, tag="ops")
                    nc.tensor.matmul(
                        out=out_ps[:],
                        lhsT=aggb[:, W * q : W * (q + 1)],
                        rhs=w_bf[:],
                        start=True,
                        stop=True,
                    )
                    o_tmp = gpool.tile([W, F], f32, tag="otmp")
                    nc.scalar.activation(
                        o_tmp[:],
                        out_ps[:],
                        mybir.ActivationFunctionType.Identity,
                        bias=0.0,
                        scale=rec[:, w : w + 1],
                    )
                    nc.vector.tensor_add(o_full[:, w, :], o_tmp[:], bias_sb[:])
                wdone = w0 + nwin
                if g == GPC - 1 and wdone > NW - 8:
                    # final graph: flush per chunk to shorten the tail
                    if wdone % WPC == 0:
                        lo = wdone - WPC
                        nc.scalar.dma_start(
                            out_d[g, :, lo:wdone, :], o_full[:, lo:wdone, :]
                        )
                elif wdone % 8 == 0:
                    lo = wdone - 8
                    nc.scalar.dma_start(
                        out_d[g, :, lo:wdone, :], o_full[:, lo:wdone, :]
                    )

            # work units: (preloaded tile or None, window base, window count)
            units = []
            if g == 0:
                units.append((at0_tiles[0], 0, 2))
                units.append((at0_tiles[1], 2, 2))
                for c in range(1, NCHUNK):
                    units.append((early_at.get(c), c * WPC, WPC))
            else:
                units = [(None, c * WPC, WPC) for c in range(NCHUNK)]

            for tile0, w0, nwin in units:
                if tile0 is None:
                    at_sb = atpool.tile([W, NW, DC], fp8, tag="at")
                    nc.sync.dma_start(at_sb[:], at_d[g, w0 // WPC])
                else:
                    at_sb = tile0
                agg_ps = apsum.tile([F, nwin * W], f32, tag=f"agg{nwin}")
                for ws in range(NW):
                    nc.tensor.matmul(
                        out=agg_ps[:],
                        lhsT=textb[:, ws, :],
                        rhs=at_sb[:, ws, :],
                        start=(ws == 0),
                        stop=(ws == NW - 1),
                    )
                aggb = gpool.tile([F, nwin * W], bf16, tag=f"aggb{nwin}")
                nc.scalar.activation(
                    aggb[:], agg_ps[:], mybir.ActivationFunctionType.Copy
                )
                pend.append((aggb, w0, nwin))
                if w0 == PREFETCH_AT * WPC and g + 1 < GPC:
                    prefetch(g + 1)
                if len(pend) > 1:
                    wapply(pend.pop(0))
            while pend:
                wapply(pend.pop(0))

    nc.compile()
    return nc


def kernel(text, weight, bias, edge_src, edge_dst):
    import ml_dtypes

    text = np.asarray(text, dtype=np.float32)
    weight = np.asarray(weight, dtype=np.float32)
    bias = np.asarray(bias, dtype=np.float32)
    edge_src = np.asarray(edge_src, dtype=np.int64)
    edge_dst = np.asarray(edge_dst, dtype=np.int64)

    if "nc" not in _cache:
        _cache["nc"] = _build_program()
    nc = _cache["nc"]

    bias_rep = np.tile(bias[None, :], (W, 1)).astype(np.float32)

    in_maps = []
    for k in range(NCORES):
        at8 = np.empty((GPC, NCHUNK, W, NW, DC), dtype=ml_dtypes.float8_e4m3)
        degw = np.empty((GPC, W, NW), dtype=np.float32)
        textw = np.empty((GPC, W, NW, F), dtype=ml_dtypes.bfloat16)
        for g in range(GPC):
            b = k * GPC + g
            src, dst = edge_src[b], edge_dst[b]
            cnt = np.bincount(src * N + dst, minlength=N * N)
            assert cnt.max() <= 15, f"edge multiplicity overflow: {cnt.max()}"
            # AT[src, dst] -> [chunk, src_row, ws, dst_col]
            at = cnt.astype(np.float32).reshape(NW, W, NCHUNK, DC)
            at8[g] = at.transpose(2, 1, 0, 3).astype(ml_dtypes.float8_e4m3)
            if g == 0:
                a0 = cnt.reshape(N, N)[:, :DC].reshape(NW, W, 2, DC // 2)
                at0 = a0.transpose(2, 1, 0, 3).astype(ml_dtypes.float8_e4m3)
            degw[g] = (
                np.bincount(dst, minlength=N).astype(np.float32).reshape(NW, W).T
            )
            textw[g] = (
                text[b].reshape(NW, W, F).transpose(1, 0, 2).astype(ml_dtypes.bfloat16)
            )
        in_maps.append(
            {
                "textw": textw,
                "weight": weight,
                "biasrep": bias_rep,
                "at8": at8,
                "at0": at0,
                "degw": degw,
            }
        )

    _cache["in_maps"] = in_maps

    from concourse.bass_utils import run_bass_kernel_spmd

    res = run_bass_kernel_spmd(nc, in_maps, list(range(NCORES)))
    # res out: [GPC, 128, NW, F] bf16 window layout -> [GPC, N, F] f32
    out = np.concatenate(
        [
            res.results[k]["out"]
            .astype(np.float32)
            .transpose(0, 2, 1, 3)
            .reshape(GPC, N, F)
            for k in range(NCORES)
        ],
        axis=0,
    )
    return np.ascontiguousarray(out)


# revision 38
# speedup vs baseline: 1.1053x; 1.1053x over previous
"""DglGraphConvolution Trainium2 kernel — dense-adjacency matmul aggregation.

out = (A @ text) @ W / (deg+1) + bias, per graph (N=4096 nodes, F=128).

Per graph:
  1. textb [128, ws, 128] bf16 (partition = node % 128 within window ws),
     DMA'd on the ACT HWDGE ring. bf16 is the kernel's internal compute
     format (the PE cannot pair an fp32 operand with the fp8 adjacency
     operand), so text is shipped already in it.
  2. Aggregation vs the dense adjacency-count matrix AT[src, dst]
     (bincount of edges, exact small ints, shipped fp8_e4m3, 2 MB
     per-chunk transfers on the SP HWDGE ring): per 512-dst chunk,
     32 accumulating matmuls
       aggT[fin, dst] += textb[:, ws, :].T @ AT[ws, chunk]  (bf16 x fp8)
     into one PSUM bank; evacuated to SBUF as bf16 (ACT).
  3. W-apply: per 128-dst window, out_ps[dst, f] = aggTb[:, w].T @ Wb
     -- output lands in natural [node, f] orientation. Emitted one chunk
     behind the aggregation so the PE never waits on the evacuation.
  4. Epilogue: ACT scales by rec[:, w] (per-partition scalar = 1/(deg+1)),
     DVE adds the replicated bias and writes bf16 into o_full.
  5. o_full DMA'd out per quarter graph (bf16); host upcasts+un-shuffles.

Host-side work is sharding plus re-encoding of the edge index lists into
adjacency counts / degree counts (np.bincount) and layout/precision
re-encodings of the model tensors into the kernel's compute formats.
"""

import numpy as np

B, N, E, F = 16, 4096, 131072, 128
NCORES = 8
GPC = B // NCORES  # graphs per core
W = 128  # node window (matmul contraction tile)
NW = N // W  # 32
DC = 512  # dst columns per chunk (one PSUM bank of f32)
NCHUNK = N // DC  # 8
WPC = DC // W  # windows per chunk = 4
PREFETCH_AT = 2  # chunk index of graph g at which graph g+1 inputs are fetched

_cache = {}


def _build_program():
    from contextlib import ExitStack

    import concourse.bacc as bacc
    import concourse.tile as tile
    from concourse import mybir
    from concourse._compat import get_trn_type

    f32 = mybir.dt.float32
    bf16 = mybir.dt.bfloat16
    fp8 = mybir.dt.float8e4

    nc = bacc.Bacc(get_trn_type() or "TRN2", target_bir_lowering=False, debug=False)

    # text in window layout: [g, p, ws, f] = text[g, ws*128+p, f], bf16
    text_d = nc.dram_tensor("textw", [GPC, W, NW, F], bf16, kind="ExternalInput")
    w_d = nc.dram_tensor("weight", [F, F], f32, kind="ExternalInput")
    bias_d = nc.dram_tensor("biasrep", [W, F], f32, kind="ExternalInput")
    # adjacency counts AT[src, dst] as [chunk, src_row, ws, dst_col]
    at_d = nc.dram_tensor("at8", [GPC, NCHUNK, W, NW, DC], fp8, kind="ExternalInput")
    # degree counts in window layout [p, w] = deg[w*128+p]
    deg_d = nc.dram_tensor("degw", [GPC, W, NW], f32, kind="ExternalInput")
    # out in window layout: [g, p, w, f] = out[g, w*128+p, f], bf16
    out_d = nc.dram_tensor("out", [GPC, W, NW, F], bf16, kind="ExternalOutput")

    with tile.TileContext(nc) as tc, ExitStack() as ctx:
        const = ctx.enter_context(tc.tile_pool(name="const", bufs=1))
        tbpool = ctx.enter_context(tc.tile_pool(name="tb", bufs=2))
        atpool = ctx.enter_context(tc.tile_pool(name="atp", bufs=6))
        gpool = ctx.enter_context(tc.tile_pool(name="gp", bufs=3))
        rpool = ctx.enter_context(tc.tile_pool(name="rp", bufs=2))
        opool = ctx.enter_context(tc.tile_pool(name="op", bufs=2))
        apsum = ctx.enter_context(tc.tile_pool(name="apsum", bufs=2, space="PSUM"))
        opsum = ctx.enter_context(tc.tile_pool(name="opsum", bufs=4, space="PSUM"))

        # Startup: both HWDGE rings carry the critical path in parallel —
        # text halves lead each ring, then the first three adjacency chunks
        # ride both rings as 1 MB halves so the chunk cadence beats the
        # PE's consumption rate while the pipeline builds its lead.
        half = NW // 2
        textb0 = tbpool.tile([W, NW, F], bf16, tag="tb")
        nc.sync.dma_start(textb0[:, :half, :], text_d[0, :, :half, :])
        nc.scalar.dma_start(textb0[:, half:, :], text_d[0, :, half:, :])
        EARLY = 3
        early_at = {}
        for c in range(EARLY):
            t = atpool.tile([W, NW, DC], fp8, tag="at")
            nc.sync.dma_start(t[:, :half, :], at_d[0, c, :, :half, :])
            nc.scalar.dma_start(t[:, half:, :], at_d[0, c, :, half:, :])
            early_at[c] = t

        w_sb = const.tile([F, F], f32)
        nc.scalar.dma_start(w_sb[:], w_d[:, :])
        w_bf = const.tile([F, F], bf16)
        nc.vector.tensor_copy(w_bf[:], w_sb[:])
        bias_sb = const.tile([W, F], f32)
        nc.scalar.dma_start(bias_sb[:], bias_d[:, :])

        # HAM pre-warm: keep the PE busy on scratch matmuls while the first
        # text/adjacency DMAs land, so the real stream starts at 2.4 GHz.
        warm_sb = const.tile([W, DC], bf16)
        nc.vector.memset(warm_sb[:], 0.0)
        warm_ps = ctx.enter_context(
            tc.tile_pool(name="warmps", bufs=1, space="PSUM")
        ).tile([F, DC], f32)
        for _ in range(36):
            nc.tensor.matmul(
                out=warm_ps[:],
                lhsT=warm_sb[:, 0:F],
                rhs=warm_sb[:],
                start=True,
                stop=True,
            )

        state = {}  # per-graph tiles, filled by prefetch

        def prefetch(g, textb=None):
            deg_sb = rpool.tile([W, NW], f32, tag="deg")
            nc.scalar.dma_start(deg_sb[:], deg_d[g])
            rec = rpool.tile([W, NW], f32, tag="rec")
            nc.vector.tensor_scalar_add(rec[:], deg_sb[:], 1.0)
            nc.vector.reciprocal(rec[:], rec[:])
            if textb is None:
                textb = tbpool.tile([W, NW, F], bf16, tag="tb")
                nc.scalar.dma_start(textb[:], text_d[g])
            state[g] = (rec, textb)

        prefetch(0, textb=textb0)
        for g in range(GPC):
            rec, textb = state.pop(g)
            o_full = opool.tile([W, NW, F], bf16, tag="of")
            pend = []  # (aggb_tile, w0, nwin)

            def wapply(entry, g=g, rec=rec, o_full=o_full):
                aggb, w0, nwin = entry
                for q in range(nwin):
                    w = w0 + q
                    out_ps = opsum.tile([W, F], f32, tag="ops")
                    nc.tensor.matmul(
                        out=out_ps[:],
                        lhsT=aggb[:, W * q : W * (q + 1)],
                        rhs=w_bf[:],
                        start=True,
                        stop=True,
                    )
                    o_tmp = gpool.tile([W, F], f32, tag="otmp")
                    nc.scalar.activation(
                        o_tmp[:],
                        out_ps[:],
                        mybir.ActivationFunctionType.Identity,
                        bias=0.0,
                        scale=rec[:, w : w + 1],
                    )
                    nc.vector.tensor_add(o_full[:, w, :], o_tmp[:], bias_sb[:])
                wdone = w0 + nwin
                if g == GPC - 1 and wdone > NW - 8:
                    # final graph: flush per chunk to shorten the tail
                    if wdone % WPC == 0:
                        lo = wdone - WPC
                        nc.scalar.dma_start(
                            out_d[g, :, lo:wdone, :], o_full[:, lo:wdone, :]
                        )
                elif wdone % 8 == 0:
                    lo = wdone - 8
                    nc.scalar.dma_start(
                        out_d[g, :, lo:wdone, :], o_full[:, lo:wdone, :]
                    )

            # work units: (preloaded tile or None, window base, window count)
            if g == 0:
                units = [(early_at.get(c), c * WPC, WPC) for c in range(NCHUNK)]
            else:
                units = [(None, c * WPC, WPC) for c in range(NCHUNK)]

            for tile0, w0, nwin in units:
                if tile0 is None:
                    at_sb = atpool.tile([W, NW, DC], fp8, tag="at")
                    nc.sync.dma_start(at_sb[:], at_d[g, w0 // WPC])
                else:
                    at_sb = tile0
                agg_ps = apsum.tile([F, nwin * W], f32, tag=f"agg{nwin}")
                for ws in range(NW):
                    nc.tensor.matmul(
                        out=agg_ps[:],
                        lhsT=textb[:, ws, :],
                        rhs=at_sb[:, ws, :],
                        start=(ws == 0),
                        stop=(ws == NW - 1),
                    )
                aggb = gpool.tile([F, nwin * W], bf16, tag=f"aggb{nwin}")
                nc.scalar.activation(
                    aggb[:], agg_ps[:], mybir.ActivationFunctionType.Copy
                )
                pend.append((aggb, w0, nwin))
                if w0 == PREFETCH_AT * WPC and g + 1 < GPC:
                    prefetch(g + 1)
                if len(pend) > 1:
                    wapply(pend.pop(0))
            while pend:
                wapply(pend.pop(0))

    nc.compile()
    return nc


def kernel(text, weight, bias, edge_src, edge_dst):
    import ml_dtypes

    text = np.asarray(text, dtype=np.float32)
    weight = np.asarray(weight, dtype=np.float32)
    bias = np.asarray(bias, dtype=np.float32)
    edge_src = np.asarray(edge_src, dtype=np.int64)
    edge_dst = np.asarray(edge_dst, dtype=np.int64)

    if "nc" not in _cache:
        _cache["nc"] = _build_program()
    nc = _cache["nc"]

    bias_rep = np.tile(bias[None, :], (W, 1)).astype(np.float32)

    in_maps = []
    for k in range(NCORES):
        at8 = np.empty((GPC, NCHUNK, W, NW, DC), dtype=ml_dtypes.float8_e4m3)
        degw = np.empty((GPC, W, NW), dtype=np.float32)
        textw = np.empty((GPC, W, NW, F), dtype=ml_dtypes.bfloat16)
        for g in range(GPC):
            b = k * GPC + g
            src, dst = edge_src[b], edge_dst[b]
            cnt = np.bincount(src * N + dst, minlength=N * N)
            assert cnt.max() <= 15, f"edge multiplicity overflow: {cnt.max()}"
            # AT[src, dst] -> [chunk, src_row, ws, dst_col]
            at = cnt.astype(np.float32).reshape(NW, W, NCHUNK, DC)
            at8[g] = at.transpose(2, 1, 0, 3).astype(ml_dtypes.float8_e4m3)
            degw[g] = (
                np.bincount(dst, minlength=N).astype(np.float32).reshape(NW, W).T
            )
            textw[g] = (
                text[b].reshape(NW, W, F).transpose(1, 0, 2).astype(ml_dtypes.bfloat16)
            )
        in_maps.append(
            {
                "textw": textw,
                "weight": weight,
                "biasrep": bias_rep,
                "at8": at8,
                "degw": degw,
            }
        )

    _cache["in_maps"] = in_maps

    from concourse.bass_utils import run_bass_kernel_spmd

    res = run_bass_kernel_spmd(nc, in_maps, list(range(NCORES)))
    # res out: [GPC, 128, NW, F] bf16 window layout -> [GPC, N, F] f32
    out = np.concatenate(
        [
            res.results[k]["out"]
            .astype(np.float32)
            .transpose(0, 2, 1, 3)
            .reshape(GPC, N, F)
            for k in range(NCORES)
        ],
        axis=0,
    )
    return np.ascontiguousarray(out)
